# revision 2
# baseline (speedup 1.0000x reference)
"""Biaffine scorer kernel for Trainium2 (Bass/Tile), data-parallel over batch
across 8 NeuronCores.

Reference computation (per batch item b):
    h = leaky_relu(state @ head_w + head_b)          # (S, BS)
    t = leaky_relu(state @ tail_w + tail_b)          # (S, BS)
    scores1[x,y,o] = h[x] @ U[o] @ t[y]
    scores2[x,y,o] = Wh.h1[x] + Wt.t1[y] + Ww.wemb[x,y] + cls_b
    out = scores1 + scores2                          # (S, S, O)

v2: the baseline was DMA-byte-bound (18.9 MB/core through the 16 SDMA
engines at ~21 GB/s each ~= 57 us busy) with the PE power-throttled to a
1.2 GHz column rate. This version halves the bytes:

  * all device tensors are bf16 (PSUM accumulation stays f32); the output
    is written bf16 and upconverted to f32 on the host. End-to-end rel err
    ~4e-3 << 2e-2 budget.
  * cls_b is folded into the ut blocks (ones-row x ones-col entry), which
    makes the width-embedding table cmat zero on its lower wedge: the
    xt=1 x-tile only needs cols 1020:2550 (the rest is a one-time memset
    + plain copies), cutting cmat DMA 2.62 MB -> 0.96 MB.
  * bias + LeakyReLU + bf16 downconvert are fused into the PSUM
    evacuation on the ACT engine (Lrelu, bias AP) - no K=1 bias matmuls.
  * finals use 510-wide chunks (51 y x 10 o rectangles of the
    [y*10+o]-flattened tUT), so the y=255 padding column is never
    computed and chunk boundaries stay rectangular for the strided APs.
  * evacuation split: DVE does all the C-add tensor_tensor ops (32/core),
    ACT does all copies/activations (32/core); both ~21 us, under the
    ~25 us DMA floor.

Device-side decomposition per pair of batch items (b0|b1 -> 512 moving):

    h1T/t1T [121, (2,256)] = Lrelu(head_w.T @ stateT, bias)   (ACT evac)
    tUT [121, (2,256,10)]  : per o, U_ext(o).T @ t1T scattered to o-strided
                             columns (ACT evac, stride-10 dst)
    out[x, (y,o)]          = h1T.T @ tUT chunk (+ cmat via DVE add evac)

cmat[x, (y*10+o)] = (width_table @ Ww.T)[pos(x,y)] is precomputed on host
(tiny) and loaded bf16. Output DMAs are one contiguous 640 KB transfer
per 128-row tile on the qSP HWDGE ring (16 descriptors x 8 rows).
"""

import numpy as np
import ml_dtypes

import concourse.bass as bass
import concourse.bacc as bacc
import concourse.tile as tile
from concourse import mybir
from concourse.bass_utils import run_bass_kernel_spmd

# problem shape (hardcoded per harness contract)
B, S, H = 32, 255, 1024
BS, WD, O = 120, 20, 10
SP = 256            # padded S
SP2 = 2 * SP        # paired moving dim
NW = SP * O         # 2560
NREAL = S * O       # 2550
KT = H // 128       # 8
NCORES = 8
BPC = B // NCORES   # 4 batch items per core
NP = BPC // 2       # 2 pairs per core
BSE = BS + 1        # 121
NC = 510            # finals chunk: 51 y x 10 o
C1LO = 2 * NC       # xt=1 tiles: first col with possible nonzero cmat
C1W = NREAL - C1LO  # 1530
C1Z = 1270 - C1LO   # 250 leading zero cols inside the cmat1 tile

F32 = mybir.dt.float32
BF16 = mybir.dt.bfloat16

_CACHE: dict = {}


def _emit(tc, d):
    """Emit the per-core program. d: dict of DRAM APs."""
    from contextlib import ExitStack

    nc = tc.nc
    AF = mybir.ActivationFunctionType
    ALU = mybir.AluOpType

    with ExitStack() as ctx:
        const = ctx.enter_context(tc.tile_pool(name="const", bufs=1))
        st_pool = ctx.enter_context(tc.tile_pool(name="st", bufs=1))
        ht_pool = ctx.enter_context(tc.tile_pool(name="ht", bufs=2))
        tut_pool = ctx.enter_context(tc.tile_pool(name="tut", bufs=2))
        out_pool = ctx.enter_context(tc.tile_pool(name="outp", bufs=3))
        pp_ht = ctx.enter_context(tc.tile_pool(name="pp_ht", bufs=1, space="PSUM"))
        pp_u = ctx.enter_context(tc.tile_pool(name="pp_u", bufs=2, space="PSUM"))
        pp_s = ctx.enter_context(tc.tile_pool(name="pp_s", bufs=4, space="PSUM"))

        # ---- persistent constants ----
        # head/tail weights carry an extra zero column (-> psum row 120 = 0);
        # biases (and the ones-row 1.0) enter via the activation bias AP.
        sb_bias = const.tile([BSE, 2], F32)
        nc.sync.dma_start(sb_bias[:], d["bias"])
        sb_hw = const.tile([128, KT * BSE], BF16)
        nc.sync.dma_start(sb_hw[:], d["hw"])
        sb_tw = const.tile([128, KT * BSE], BF16)
        nc.sync.dma_start(sb_tw[:], d["tw"])
        # ut: per-o [121, 121] blocks (Wt/Wh/cls_b folded in).
        sb_ut = const.tile([BSE, O * BSE], BF16)
        nc.sync.dma_start(sb_ut[:], d["ut"])
        sb_c0 = const.tile([128, NREAL], BF16)
        sb_c1 = const.tile([128, C1W], BF16)

        # stateT for both pairs, issued upfront on the scalar ring (four
        # 128-row HWDGE reads spread across the SDMA engines).
        half = KT * SP2 // 2
        sb_sT = []
        for p in range(NP):
            a = st_pool.tile([128, half], BF16)
            b = st_pool.tile([128, half], BF16)
            nc.scalar.dma_start(a[:], d["stateT"][p][:, 0:half])
            nc.scalar.dma_start(b[:], d["stateT"][p][:, half:])
            sb_sT.append((a, b))

        # cmat rides the qSP ring behind the small consts; first needed by
        # the first finals (~10us in).
        nc.sync.dma_start(sb_c0[:], d["cmat0"])
        nc.sync.dma_start(sb_c1[:, C1Z:], d["cmat1"])
        # leading zero wedge of the xt=1 cmat tile (Pool engine is idle)
        nc.gpsimd.memset(sb_c1[:, 0:C1Z], 0.0)

        for p in range(NP):
            sb_sTa, sb_sTb = sb_sT[p]

            # ---- head/tail projections -> h1T/t1T [121, (2,256)] ----
            ps_h = pp_ht.tile([BSE, 2, SP], F32)
            ps_t = pp_ht.tile([BSE, 2, SP], F32)
            for ps, w in ((ps_h, sb_hw), (ps_t, sb_tw)):
                for kt in range(KT):
                    st = sb_sTa if kt < 4 else sb_sTb
                    nc.tensor.matmul(
                        ps[:, :, :],
                        lhsT=w[:, kt * BSE:(kt + 1) * BSE],
                        rhs=st[:, (kt % 4) * SP2:(kt % 4 + 1) * SP2],
                        start=(kt == 0),
                        stop=(kt == KT - 1),
                    )
            # fused evac: bf16 <- Lrelu(psum + bias); psum row 120 is 0 and
            # bias row 120 is 1.0 -> ones row.
            h1T = ht_pool.tile([BSE, 2, SP], BF16)
            t1T = ht_pool.tile([BSE, 2, SP], BF16)
            nc.scalar.activation(
                h1T[:, :, :], ps_h[:, :, :], AF.Lrelu,
                bias=sb_bias[:, 0:1], alpha=0.01,
            )
            nc.scalar.activation(
                t1T[:, :, :], ps_t[:, :, :], AF.Lrelu,
                bias=sb_bias[:, 1:2], alpha=0.01,
            )

            # ---- tUT [121, (2, 256, 10)]: per-o strided scatter evac ----
            tUT = tut_pool.tile([BSE, 2, SP, O], BF16)
            for o in range(O):
                ps_u = pp_u.tile([BSE, 2, SP], F32)
                nc.tensor.matmul(
                    ps_u[:, :, :],
                    lhsT=sb_ut[:, o * BSE:(o + 1) * BSE],
                    rhs=t1T[:, :, :],
                    start=True,
                    stop=True,
                )
                nc.scalar.activation(tUT[:, :, :, o], ps_u[:, :, :], AF.Copy)

            # ---- finals: out[x, (y,o)] per (b-in-pair, x-tile) ----
            for bb in range(2):
                for xt in range(2):
                    sb_out = out_pool.tile([128, NW], BF16)
                    for c in range(5):
                        ps_s = pp_s.tile([128, NC], F32)
                        nc.tensor.matmul(
                            ps_s[:, :],
                            lhsT=h1T[:, bb, xt * 128:(xt + 1) * 128],
                            rhs=tUT[:, bb, 51 * c:51 * (c + 1), :],
                            start=True,
                            stop=True,
                        )
                        oc = sb_out[:, NC * c:NC * (c + 1)]
                        if xt == 0:
                            nc.vector.tensor_tensor(
                                oc, ps_s[:, :], sb_c0[:, NC * c:NC * (c + 1)],
                                op=ALU.add,
                            )
                        elif c < 2:
                            # cmat is identically zero here (y <= x-2)
                            nc.scalar.activation(oc, ps_s[:, :], AF.Copy)
                        else:
                            nc.vector.tensor_tensor(
                                oc, ps_s[:, :],
                                sb_c1[:, NC * c - C1LO:NC * (c + 1) - C1LO],
                                op=ALU.add,
                            )
                    # cols 2550:2560 (y=255) are never computed; give the
                    # DMA defined bytes (host slices them off).
                    nc.gpsimd.memset(sb_out[:, NREAL:NW], 0.0)
                    nc.sync.dma_start(
                        d["out"][2 * p + bb, xt * 128:(xt + 1) * 128, :],
                        sb_out[:],
                    )


def build_nc():
    if "nc" in _CACHE:
        return _CACHE["nc"]
    nc = bacc.Bacc(
        "TRN2", target_bir_lowering=False, debug=False, num_devices=NCORES
    )
    d = {}
    d["stateT"] = nc.dram_tensor(
        "stateT", [NP, 128, KT * SP2], BF16, kind="ExternalInput"
    ).ap()
    d["hw"] = nc.dram_tensor("hw", [128, KT * BSE], BF16, kind="ExternalInput").ap()
    d["tw"] = nc.dram_tensor("tw", [128, KT * BSE], BF16, kind="ExternalInput").ap()
    d["ut"] = nc.dram_tensor("ut", [BSE, O * BSE], BF16, kind="ExternalInput").ap()
    d["bias"] = nc.dram_tensor("bias", [BSE, 2], F32, kind="ExternalInput").ap()
    d["cmat0"] = nc.dram_tensor("cmat0", [128, NREAL], BF16, kind="ExternalInput").ap()
    d["cmat1"] = nc.dram_tensor("cmat1", [128, C1W - C1Z], BF16, kind="ExternalInput").ap()
    d["out"] = nc.dram_tensor("out", [BPC, SP, NW], BF16, kind="ExternalOutput").ap()

    with tile.TileContext(nc) as tc:
        _emit(tc, d)
    nc.compile()
    _CACHE["nc"] = nc
    return nc


def prep_inputs(inputs):
    """Host-side constant packing + state transpose. Returns dict of np arrays
    shared across cores (stateT is full-batch; shard before dispatch)."""
    bf16 = ml_dtypes.bfloat16
    state = np.asarray(inputs["state"], np.float32)
    head_w = np.asarray(inputs["head_w"], np.float32)
    head_b = np.asarray(inputs["head_b"], np.float32)
    tail_w = np.asarray(inputs["tail_w"], np.float32)
    tail_b = np.asarray(inputs["tail_b"], np.float32)
    U = np.asarray(inputs["U"], np.float32)
    width_table = np.asarray(inputs["width_table"], np.float32)
    cls_w = np.asarray(inputs["cls_w"], np.float32)
    cls_b = np.asarray(inputs["cls_b"], np.float32)

    # stateT paired pack: [B/2, 128, (kt, b01, y)], y zero-padded to 256
    stateT = np.zeros((B, H, SP), np.float32)
    stateT[:, :, :S] = state.transpose(0, 2, 1)
    # [B/2, 2, KT, 128, SP] -> [B/2, 128, KT, 2, SP]
    stateT = stateT.reshape(B // 2, 2, KT, 128, SP).transpose(0, 3, 2, 1, 4)
    stateT = np.ascontiguousarray(stateT.reshape(B // 2, 128, KT * SP2)).astype(bf16)

    hw_sb = np.zeros((128, KT, BSE), np.float32)
    hw_sb[:, :, :BS] = head_w.reshape(KT, 128, BS).transpose(1, 0, 2)
    hw_sb = hw_sb.reshape(128, KT * BSE).astype(bf16)
    tw_sb = np.zeros((128, KT, BSE), np.float32)
    tw_sb[:, :, :BS] = tail_w.reshape(KT, 128, BS).transpose(1, 0, 2)
    tw_sb = tw_sb.reshape(128, KT * BSE).astype(bf16)

    # ut blocks: [j, o, i] = U[o, i, j]; Wt in col i=BS; Wh folded into the
    # ones-row j=BS; cls_b folded into the (j=BS, i=BS) corner.
    blocks = np.zeros((BSE, O, BSE), np.float32)
    blocks[:BS, :, :BS] = U.transpose(2, 0, 1)
    blocks[:, :, BS] = cls_w[:, BS + 1:2 * (BS + 1)].T
    blocks[BS, :, :] += cls_w[:, :BSE]
    blocks[BS, :, BS] += cls_b
    ut = blocks.reshape(BSE, O * BSE).astype(bf16)

    bias = np.zeros((BSE, 2), np.float32)
    bias[:BS, 0] = head_b
    bias[:BS, 1] = tail_b
    bias[BS, :] = 1.0

    # cmat (cls_b excluded -> zero lower wedge): cvals[pos(x,y)] @ Ww.T
    pos = np.arange(S)[None, :] - np.arange(S)[:, None] + 1
    pos = pos * (pos > 0)
    cvals = width_table @ cls_w[:, 2 * (BS + 1):].T        # [256, 10], row0=0
    cfull = cvals[pos].reshape(S, NREAL).astype(bf16)       # [255, 2550]
    cmat0 = np.zeros((128, NREAL), bf16)
    cmat0[:] = cfull[:128]
    cmat1 = np.zeros((128, C1W - C1Z), bf16)
    cmat1[:S - 128] = cfull[128:, 1270:]

    return {
        "stateT": stateT,
        "hw": hw_sb,
        "tw": tw_sb,
        "ut": ut,
        "bias": bias,
        "cmat0": cmat0,
        "cmat1": cmat1,
    }


def run(inputs, trace=False, trace_kwargs=None):
    nc = build_nc()
    full = prep_inputs(inputs)
    shared = {k: v for k, v in full.items() if k != "stateT"}
    in_maps = []
    for c in range(NCORES):
        m = dict(shared)
        m["stateT"] = np.ascontiguousarray(full["stateT"][c * NP:(c + 1) * NP])
        in_maps.append(m)
    res = run_bass_kernel_spmd(
        nc,
        in_maps,
        core_ids=list(range(NCORES)),
        trace=trace,
        **(trace_kwargs or {}),
    )
    out = np.concatenate(
        [np.asarray(r["out"]).astype(np.float32) for r in res.results], axis=0
    )
    out = out[:, :S, :NREAL].reshape(B, S, S, O)
    return out, res


def kernel(**inputs):
    out, _ = run(inputs, trace=False)
    return out


if __name__ == "__main__":
    build_nc()
    print("build ok")


# revision 7
# speedup vs baseline: 1.3027x; 1.3027x over previous
"""Biaffine scorer kernel for Trainium2 (Bass/Tile), data-parallel over batch
across 8 NeuronCores.

Reference computation (per batch item b):
    h = leaky_relu(state @ head_w + head_b)          # (S, BS)
    t = leaky_relu(state @ tail_w + tail_b)          # (S, BS)
    scores1[x,y,o] = h[x] @ U[o] @ t[y]
    scores2[x,y,o] = Wh.h1[x] + Wt.t1[y] + Ww.wemb[x,y] + cls_b
    out = scores1 + scores2                          # (S, S, O)

All device tensors are bf16 (PSUM accumulation stays f32); the output is
written bf16 and upconverted to f32 on the host. End-to-end rel err ~5e-3
against the f32 reference (budget 2e-2). This halves the dominant DMA
traffic (9.1 MB/core vs 18.9 f32) so the 16 SDMA engines (~21 GB/s each)
stop being the bottleneck.

Key structure choices (each one measured against a trace):

  * tUT is stored per-o CONTIGUOUS ([121, (bb, o, y)]) and the finals
    matmul reads the (y*10+o)-interleaved columns through a transposed
    2-dim rhs access pattern (51 y x 10 o rectangles, stride (1, 256)).
    Writing tUT o-interleaved instead (stride-10 bf16 ACT writes) runs at
    ~5.5 cycles/elem on the ACT engine - 4x slower than contiguous.
  * cls_b is folded into the ut blocks (ones-row x ones-col entry), so the
    width-embedding table cmat is zero on its lower wedge: x-tile 1 only
    loads cols 1020:2550 and its first two chunks are plain copies.
  * bias + LeakyReLU + bf16 downconvert are fused into the PSUM
    evacuation on the ACT engine (Lrelu with a bias column AP, alpha=.01);
    the psum row 120 is 0 and bias row 120 is 1.0 -> the ones feature.
  * finals chunks are 510 wide (51 y x 10 o), so the y=255 padding column
    is never computed and chunk boundaries stay rectangular.
  * evacuation split: DVE does the 32 cmat tensor_tensor adds, ACT the 32
    copies/activations; ~21 us each, under the PE's ~40 us.
  * stateT arrives as 16 per-kt 128 KB contiguous chunks (pair 0 on the
    scalar ring, pair 1 on the vector ring) and the projection matmuls
    interleave h/t per kt, so the first matmul can start ~7 us earlier
    than with half-tensor loads.
  * pair pipeline A(0) A(1) B(0) B(1) (A = proj+tUT build, B = finals):
    pair 1's projections fill the PE bubble while pair 0's tUT finishes
    evacuating, keeping the PE continuously busy (the DVFS governor
    parks the PE at 4/8 duty when it idles).
"""

import numpy as np
import ml_dtypes

import concourse.bass as bass
import concourse.bacc as bacc
import concourse.tile as tile
from concourse import mybir
from concourse.bass_utils import run_bass_kernel_spmd

# problem shape (hardcoded per harness contract)
B, S, H = 32, 255, 1024
BS, WD, O = 120, 20, 10
SP = 256            # padded S
SP2 = 2 * SP        # paired moving dim
NW = SP * O         # 2560
NREAL = S * O       # 2550
KT = H // 128       # 8
NCORES = 8
BPC = B // NCORES   # 4 batch items per core
NP = BPC // 2       # 2 pairs per core
BSE = BS + 1        # 121
NC = 510            # finals chunk: 51 y x 10 o
C1LO = 2 * NC       # xt=1 tiles: first col with possible nonzero cmat
C1W = NREAL - C1LO  # 1530
C1Z = 1270 - C1LO   # 250 leading zero cols inside the cmat1 tile

F32 = mybir.dt.float32
BF16 = mybir.dt.bfloat16

_CACHE: dict = {}


def _emit(tc, d):
    """Emit the per-core program. d: dict of DRAM APs."""
    from contextlib import ExitStack

    nc = tc.nc
    AF = mybir.ActivationFunctionType
    ALU = mybir.AluOpType

    with ExitStack() as ctx:
        const = ctx.enter_context(tc.tile_pool(name="const", bufs=1))
        st_pool = ctx.enter_context(tc.tile_pool(name="st", bufs=1))
        ht_pool = ctx.enter_context(tc.tile_pool(name="ht", bufs=2))
        tut_pool = ctx.enter_context(tc.tile_pool(name="tut", bufs=2))
        out_pool = ctx.enter_context(tc.tile_pool(name="outp", bufs=3))
        pp_ht = ctx.enter_context(tc.tile_pool(name="pp_ht", bufs=1, space="PSUM"))
        pp_u = ctx.enter_context(tc.tile_pool(name="pp_u", bufs=2, space="PSUM"))
        pp_s = ctx.enter_context(tc.tile_pool(name="pp_s", bufs=4, space="PSUM"))

        # ---- persistent constants (qSP ring) ----
        # head/tail weights carry an extra zero column (-> psum row 120 = 0);
        # biases (and the ones-row 1.0) enter via the activation bias AP.
        sb_bias = const.tile([BSE, 2], F32)
        nc.sync.dma_start(sb_bias[:], d["bias"])
        sb_hw = const.tile([128, KT * BSE], BF16)
        nc.sync.dma_start(sb_hw[:], d["hw"])
        sb_tw = const.tile([128, KT * BSE], BF16)
        nc.sync.dma_start(sb_tw[:], d["tw"])
        # ut: per-o [121, 121] blocks (Wt/Wh/cls_b folded in).
        sb_ut = const.tile([BSE, O * BSE], BF16)
        nc.sync.dma_start(sb_ut[:], d["ut"])
        sb_c0 = const.tile([128, 5, NC], BF16)
        sb_c1 = const.tile([128, 3, NC], BF16)

        # stateT: 16 contiguous 128 KB per-kt chunks; pair 0 rides the
        # scalar ring (needed first), pair 1 the vector ring.
        sb_sT = [
            [
                st_pool.tile([128, SP2], BF16, name=f"sT_{p}_{kt}")
                for kt in range(KT)
            ]
            for p in range(NP)
        ]
        for kt in range(KT):
            nc.scalar.dma_start(sb_sT[0][kt][:], d["stateT"][0, kt])
        for kt in range(KT):
            nc.sync.dma_start(sb_sT[1][kt][:], d["stateT"][1, kt])

        # cmat rides the qSP ring behind pair 1's state; first needed by
        # the first finals.
        nc.sync.dma_start(sb_c0[:], d["cmat0"])
        nc.sync.dma_start(sb_c1[:], d["cmat1"])

        hts, tuts = [], []

        def stage_a(p):
            # ---- head/tail projections -> h1T/t1T [121, (2,256)] ----
            ps_h = pp_ht.tile([BSE, 2, SP], F32)
            ps_t = pp_ht.tile([BSE, 2, SP], F32)
            for kt in range(KT):
                for ps, w in ((ps_h, sb_hw), (ps_t, sb_tw)):
                    nc.tensor.matmul(
                        ps[:, :, :],
                        lhsT=w[:, kt * BSE:(kt + 1) * BSE],
                        rhs=sb_sT[p][kt][:],
                        start=(kt == 0),
                        stop=(kt == KT - 1),
                    )
            # fused evac: bf16 <- Lrelu(psum + bias); psum row 120 is 0 and
            # bias row 120 is 1.0 -> ones row.
            h1T = ht_pool.tile([BSE, 2, SP], BF16)
            t1T = ht_pool.tile([BSE, 2, SP], BF16)
            nc.scalar.activation(
                h1T[:, :, :], ps_h[:, :, :], AF.Lrelu,
                bias=sb_bias[:, 0:1], alpha=0.01,
            )
            nc.scalar.activation(
                t1T[:, :, :], ps_t[:, :, :], AF.Lrelu,
                bias=sb_bias[:, 1:2], alpha=0.01,
            )

            # ---- tUT [121, (2, 10, 256)]: contiguous per-o evac ----
            tUT = tut_pool.tile([BSE, 2, O, SP], BF16)
            for o in range(O):
                ps_u = pp_u.tile([BSE, 2, SP], F32)
                nc.tensor.matmul(
                    ps_u[:, :, :],
                    lhsT=sb_ut[:, o * BSE:(o + 1) * BSE],
                    rhs=t1T[:, :, :],
                    start=True,
                    stop=True,
                )
                nc.scalar.activation(tUT[:, :, o, :], ps_u[:, :, :], AF.Copy)
            hts.append(h1T)
            tuts.append(tUT)

        def stage_b(p, last):
            h1T, tUT = hts[p], tuts[p]
            for bb in range(2):
                for xt in range(2):
                    is_last = last and bb == 1 and xt == 1
                    sb_out = out_pool.tile([128, NW], BF16)
                    for c in range(5):
                        ps_s = pp_s.tile([128, NC], F32)
                        # rhs: 51 y x 10 o rectangle of the (y*10+o)-
                        # flattened tUT, via a transposed strided AP.
                        nc.tensor.matmul(
                            ps_s[:, :],
                            lhsT=h1T[:, bb, xt * 128:(xt + 1) * 128],
                            rhs=tUT[:, bb, :, 51 * c:51 * (c + 1)].transpose(
                                [0, 2, 1]
                            ),
                            start=True,
                            stop=True,
                        )
                        oc = sb_out[:, NC * c:NC * (c + 1)]
                        if xt == 0:
                            nc.vector.tensor_tensor(
                                oc, ps_s[:, :], sb_c0[:, c, :], op=ALU.add
                            )
                        elif c < 2:
                            # cmat is identically zero here (y <= x-2)
                            nc.scalar.activation(oc, ps_s[:, :], AF.Copy)
                        else:
                            nc.vector.tensor_tensor(
                                oc, ps_s[:, :], sb_c1[:, c - 2, :], op=ALU.add
                            )
                        if is_last and c == 2:
                            # tail: ship the finished two-thirds early
                            nc.sync.dma_start(
                                d["out"][2 * p + bb, xt * 128:(xt + 1) * 128,
                                         0:3 * NC],
                                sb_out[:, 0:3 * NC],
                            )
                    # cols 2550:2560 (y=255) are never computed; give the
                    # DMA defined bytes (host slices them off).
                    nc.gpsimd.memset(sb_out[:, NREAL:NW], 0.0)
                    if is_last:
                        nc.sync.dma_start(
                            d["out"][2 * p + bb, xt * 128:(xt + 1) * 128,
                                     3 * NC:],
                            sb_out[:, 3 * NC:],
                        )
                    else:
                        nc.sync.dma_start(
                            d["out"][2 * p + bb, xt * 128:(xt + 1) * 128, :],
                            sb_out[:],
                        )

        # software pipeline: A(0) A(1) B(0) B(1) keeps the PE busy while
        # pair 0's tUT finishes evacuating on the ACT engine.
        stage_a(0)
        stage_a(1)
        stage_b(0, last=False)
        stage_b(1, last=True)


def build_nc():
    if "nc" in _CACHE:
        return _CACHE["nc"]
    nc = bacc.Bacc(
        "TRN2", target_bir_lowering=False, debug=False, num_devices=NCORES
    )
    d = {}
    d["stateT"] = nc.dram_tensor(
        "stateT", [NP, KT, 128, SP2], BF16, kind="ExternalInput"
    ).ap()
    d["hw"] = nc.dram_tensor("hw", [128, KT * BSE], BF16, kind="ExternalInput").ap()
    d["tw"] = nc.dram_tensor("tw", [128, KT * BSE], BF16, kind="ExternalInput").ap()
    d["ut"] = nc.dram_tensor("ut", [BSE, O * BSE], BF16, kind="ExternalInput").ap()
    d["bias"] = nc.dram_tensor("bias", [BSE, 2], F32, kind="ExternalInput").ap()
    d["cmat0"] = nc.dram_tensor(
        "cmat0", [128, 5, NC], BF16, kind="ExternalInput"
    ).ap()
    d["cmat1"] = nc.dram_tensor(
        "cmat1", [128, 3, NC], BF16, kind="ExternalInput"
    ).ap()
    d["out"] = nc.dram_tensor("out", [BPC, SP, NW], BF16, kind="ExternalOutput").ap()

    with tile.TileContext(nc) as tc:
        _emit(tc, d)
    nc.compile()
    _CACHE["nc"] = nc
    return nc


def prep_inputs(inputs):
    """Host-side constant packing + state transpose. Returns dict of np arrays
    shared across cores (stateT is full-batch; shard before dispatch)."""
    bf16 = ml_dtypes.bfloat16
    state = np.asarray(inputs["state"], np.float32)
    head_w = np.asarray(inputs["head_w"], np.float32)
    head_b = np.asarray(inputs["head_b"], np.float32)
    tail_w = np.asarray(inputs["tail_w"], np.float32)
    tail_b = np.asarray(inputs["tail_b"], np.float32)
    U = np.asarray(inputs["U"], np.float32)
    width_table = np.asarray(inputs["width_table"], np.float32)
    cls_w = np.asarray(inputs["cls_w"], np.float32)
    cls_b = np.asarray(inputs["cls_b"], np.float32)

    # stateT pack: [B/2, KT, 128, (b01, y)], y zero-padded to 256
    stateT = np.zeros((B, H, SP), np.float32)
    stateT[:, :, :S] = state.transpose(0, 2, 1)
    # [B/2, 2, KT, 128, SP] -> [B/2, KT, 128, 2, SP]
    stateT = stateT.reshape(B // 2, 2, KT, 128, SP).transpose(0, 2, 3, 1, 4)
    stateT = np.ascontiguousarray(
        stateT.reshape(B // 2, KT, 128, SP2)
    ).astype(bf16)

    hw_sb = np.zeros((128, KT, BSE), np.float32)
    hw_sb[:, :, :BS] = head_w.reshape(KT, 128, BS).transpose(1, 0, 2)
    hw_sb = hw_sb.reshape(128, KT * BSE).astype(bf16)
    tw_sb = np.zeros((128, KT, BSE), np.float32)
    tw_sb[:, :, :BS] = tail_w.reshape(KT, 128, BS).transpose(1, 0, 2)
    tw_sb = tw_sb.reshape(128, KT * BSE).astype(bf16)

    # ut blocks: [j, o, i] = U[o, i, j]; Wt in col i=BS; Wh folded into the
    # ones-row j=BS; cls_b folded into the (j=BS, i=BS) corner.
    blocks = np.zeros((BSE, O, BSE), np.float32)
    blocks[:BS, :, :BS] = U.transpose(2, 0, 1)
    blocks[:, :, BS] = cls_w[:, BS + 1:2 * (BS + 1)].T
    blocks[BS, :, :] += cls_w[:, :BSE]
    blocks[BS, :, BS] += cls_b
    ut = blocks.reshape(BSE, O * BSE).astype(bf16)

    bias = np.zeros((BSE, 2), np.float32)
    bias[:BS, 0] = head_b
    bias[:BS, 1] = tail_b
    bias[BS, :] = 1.0

    # cmat (cls_b excluded -> zero lower wedge): cvals[pos(x,y)] @ Ww.T
    pos = np.arange(S)[None, :] - np.arange(S)[:, None] + 1
    pos = pos * (pos > 0)
    cvals = width_table @ cls_w[:, 2 * (BS + 1):].T        # [256, 10], row0=0
    cfull = cvals[pos].reshape(S, NREAL).astype(bf16)       # [255, 2550]
    cmat0 = cfull[:128].reshape(128, 5, NC)
    cmat1 = np.zeros((128, 3 * NC), bf16)
    cmat1[:S - 128, C1Z:] = cfull[128:, 1270:]
    cmat1 = np.ascontiguousarray(cmat1.reshape(128, 3, NC))

    return {
        "stateT": stateT,
        "hw": hw_sb,
        "tw": tw_sb,
        "ut": ut,
        "bias": bias,
        "cmat0": np.ascontiguousarray(cmat0),
        "cmat1": cmat1,
    }


def run(inputs, trace=False, trace_kwargs=None):
    nc = build_nc()
    full = prep_inputs(inputs)
    shared = {k: v for k, v in full.items() if k != "stateT"}
    in_maps = []
    for c in range(NCORES):
        m = dict(shared)
        m["stateT"] = np.ascontiguousarray(full["stateT"][c * NP:(c + 1) * NP])
        in_maps.append(m)
    res = run_bass_kernel_spmd(
        nc,
        in_maps,
        core_ids=list(range(NCORES)),
        trace=trace,
        **(trace_kwargs or {}),
    )
    out = np.concatenate(
        [np.asarray(r["out"]).astype(np.float32) for r in res.results], axis=0
    )
    out = out[:, :S, :NREAL].reshape(B, S, S, O)
    return out, res


def kernel(**inputs):
    out, _ = run(inputs, trace=False)
    return out


if __name__ == "__main__":
    build_nc()
    print("build ok")


# revision 9
# speedup vs baseline: 1.7561x; 1.3480x over previous
"""Biaffine scorer kernel for Trainium2 (Bass/Tile), data-parallel over batch
across 8 NeuronCores.

Reference computation (per batch item b):
    h = leaky_relu(state @ head_w + head_b)          # (S, BS)
    t = leaky_relu(state @ tail_w + tail_b)          # (S, BS)
    scores1[x,y,o] = h[x] @ U[o] @ t[y]
    scores2[x,y,o] = Wh.h1[x] + Wt.t1[y] + Ww.wemb[x,y] + cls_b
    out = scores1 + scores2                          # (S, S, O)

All device tensors are bf16 (PSUM accumulation stays f32); the output is
written bf16 and upconverted on the host. End-to-end rel err ~5e-3 against
the f32 reference (budget 2e-2). bf16 halves the dominant DMA traffic
(9.5 MB/core vs 18.9 f32) so the 16 SDMA engines stop being the
bottleneck.

Key structure choices (each one measured against a trace):

  * The device output layout is [b][x][o][y]; the host transposes (o,y) ->
    (y,o) while upconverting. This keeps BOTH sides of the finals matmul
    contiguous: an o-interleaved SBUF layout needs either stride-10 bf16
    ACT writes (~5.5 cyc/elem, 4x slow) or a transposed strided matmul
    rhs AP (2 PE cycles/col, 2x slow). Measured both; contiguous wins.
  * cls_b is folded into the ut blocks (ones-row x ones-col entry), so
    the width-embedding cmat term is zero on the y <= x-2 wedge: for the
    x>=128 tile each chunk splits into an ACT copy (y<127, cmat==0) and a
    DVE add (y>=127), balancing the two evacuation engines.
  * bias + LeakyReLU + bf16 downconvert are fused into the PSUM
    evacuation on the ACT engine (Lrelu with a bias column AP, alpha=.01);
    psum row 120 is 0 and bias row 120 is 1.0 -> the ones feature.
  * stateT arrives as 16 per-kt 128 KB contiguous chunks (pair 0 on the
    scalar ring - idle until the first Lrelu - pair 1 on the qSP ring),
    so the first projection matmul starts as early as possible.
  * pair pipeline: A(0), proj(1), then B(0) finals interleaved with
    pair 1's tUT build, then B(1). The PE's HAM clock gate parks the
    array at 4/8 duty (1.2 GHz) whenever it idles a ~3.4us window, so
    the PE stream must never stall: pair 1's work fills the gap while
    pair 0's tUT evacuates, and B(0)'s DVE adds overlap pair 1's PE time.

Per-pair device decomposition (pair = batch items b0|b1, 512 moving):

    h1T/t1T [121, (2,256)] = Lrelu(head_w.T @ stateT + bias)  (ACT evac)
    tUT [121, (2, 10, 256)]: per o, U_ext(o).T @ t1T           (ACT evac)
    out[x, (o,y)] chunks    = h1T.T @ tUT[:, bb, 2c:2c+2, :]  (+cmat, DVE)
"""

import numpy as np
import ml_dtypes

import concourse.bass as bass
import concourse.bacc as bacc
import concourse.tile as tile
from concourse import mybir
from concourse.bass_utils import run_bass_kernel_spmd

# problem shape (hardcoded per harness contract)
B, S, H = 32, 255, 1024
BS, WD, O = 120, 20, 10
SP = 256            # padded S
SP2 = 2 * SP        # paired moving dim
NW = SP * O         # 2560
KT = H // 128       # 8
NCORES = 8
BPC = B // NCORES   # 4 batch items per core
NP = BPC // 2       # 2 pairs per core
BSE = BS + 1        # 121
YZ = 127            # xt=1 tiles: cmat is zero for y < YZ

F32 = mybir.dt.float32
BF16 = mybir.dt.bfloat16

_CACHE: dict = {}


def _emit(tc, d):
    """Emit the per-core program. d: dict of DRAM APs."""
    from contextlib import ExitStack

    nc = tc.nc
    AF = mybir.ActivationFunctionType
    ALU = mybir.AluOpType

    with ExitStack() as ctx:
        const = ctx.enter_context(tc.tile_pool(name="const", bufs=1))
        st_pool = ctx.enter_context(tc.tile_pool(name="st", bufs=1))
        ht_pool = ctx.enter_context(tc.tile_pool(name="ht", bufs=1))
        tut_pool = ctx.enter_context(tc.tile_pool(name="tut", bufs=1))
        out_pool = ctx.enter_context(tc.tile_pool(name="outp", bufs=3))
        pp_ht = ctx.enter_context(tc.tile_pool(name="pp_ht", bufs=1, space="PSUM"))
        pp_u = ctx.enter_context(tc.tile_pool(name="pp_u", bufs=2, space="PSUM"))
        pp_s = ctx.enter_context(tc.tile_pool(name="pp_s", bufs=4, space="PSUM"))

        # ---- persistent constants (qSP ring) ----
        # head/tail weights carry an extra zero column (-> psum row 120 = 0);
        # biases (and the ones-row 1.0) enter via the activation bias AP.
        sb_bias = const.tile([BSE, 2], F32)
        nc.sync.dma_start(sb_bias[:], d["bias"])
        sb_hw = const.tile([128, KT * BSE], BF16)
        nc.sync.dma_start(sb_hw[:], d["hw"])
        sb_tw = const.tile([128, KT * BSE], BF16)
        nc.sync.dma_start(sb_tw[:], d["tw"])
        # ut: per-o [121, 121] blocks (Wt/Wh/cls_b folded in).
        sb_ut = const.tile([BSE, O * BSE], BF16)
        nc.sync.dma_start(sb_ut[:], d["ut"])
        sb_c0 = const.tile([128, O, SP], BF16)
        sb_c1 = const.tile([128, O, SP], BF16)

        # stateT: 16 contiguous 128 KB per-kt chunks; pair 0 rides the
        # scalar ring (ACT is idle until the first Lrelu), pair 1 + cmat
        # follow the consts on the qSP ring.
        sb_sT = [
            [
                st_pool.tile([128, SP2], BF16, name=f"sT_{p}_{kt}")
                for kt in range(KT)
            ]
            for p in range(NP)
        ]
        for kt in range(KT):
            nc.scalar.dma_start(sb_sT[0][kt][:], d["stateT"][0, kt])
        for kt in range(KT):
            nc.sync.dma_start(sb_sT[1][kt][:], d["stateT"][1, kt])
        nc.sync.dma_start(sb_c0[:], d["cmat0"])
        nc.sync.dma_start(sb_c1[:], d["cmat1"])

        hts, tuts = [], []

        def proj(p):
            # ---- head/tail projections -> h1T/t1T [121, (2,256)] ----
            ps_h = pp_ht.tile([BSE, 2, SP], F32, name="ps_h")
            ps_t = pp_ht.tile([BSE, 2, SP], F32, name="ps_t")
            for kt in range(KT):
                for ps, w in ((ps_h, sb_hw), (ps_t, sb_tw)):
                    nc.tensor.matmul(
                        ps[:, :, :],
                        lhsT=w[:, kt * BSE:(kt + 1) * BSE],
                        rhs=sb_sT[p][kt][:],
                        start=(kt == 0),
                        stop=(kt == KT - 1),
                    )
            # fused evac: bf16 <- Lrelu(psum + bias)
            h1T = ht_pool.tile([BSE, 2, SP], BF16, name=f"h1T{p}")
            t1T = ht_pool.tile([BSE, 2, SP], BF16, name=f"t1T{p}")
            nc.scalar.activation(
                h1T[:, :, :], ps_h[:, :, :], AF.Lrelu,
                bias=sb_bias[:, 0:1], alpha=0.01,
            )
            nc.scalar.activation(
                t1T[:, :, :], ps_t[:, :, :], AF.Lrelu,
                bias=sb_bias[:, 1:2], alpha=0.01,
            )
            hts.append(h1T)
            tut = tut_pool.tile([BSE, 2, O, SP], BF16, name=f"tUT{p}")
            tuts.append(tut)
            return t1T

        def tut_step(p, t1T, o):
            # tUT[:, :, o, :] <- U_ext(o).T @ t1T   (contiguous ACT evac)
            ps_u = pp_u.tile([BSE, 2, SP], F32, name="ps_u")
            nc.tensor.matmul(
                ps_u[:, :, :],
                lhsT=sb_ut[:, o * BSE:(o + 1) * BSE],
                rhs=t1T[:, :, :],
                start=True,
                stop=True,
            )
            nc.scalar.activation(
                tuts[p][:, :, o, :], ps_u[:, :, :], AF.Copy
            )

        def final_chunk(p, bb, xt, c, sb_out):
            # out[x, (2 o, 256 y)] = h1T.T @ tUT chunk, + cmat on evac
            ps_s = pp_s.tile([128, 2, SP], F32, name="ps_s")
            nc.tensor.matmul(
                ps_s[:, :, :],
                lhsT=hts[p][:, bb, xt * 128:(xt + 1) * 128],
                rhs=tuts[p][:, bb, 2 * c:2 * c + 2, :],
                start=True,
                stop=True,
            )
            oc = sb_out[:, 2 * c:2 * c + 2, :]
            if xt == 0:
                nc.vector.tensor_tensor(
                    oc, ps_s[:, :, :], sb_c0[:, 2 * c:2 * c + 2, :], op=ALU.add
                )
            else:
                # cmat is zero for y < 127 on the x>=128 tile: split the
                # evacuation into an ACT copy and a DVE add.
                nc.scalar.activation(
                    oc[:, :, 0:YZ], ps_s[:, :, 0:YZ], AF.Copy
                )
                nc.vector.tensor_tensor(
                    oc[:, :, YZ:], ps_s[:, :, YZ:],
                    sb_c1[:, 2 * c:2 * c + 2, YZ:], op=ALU.add,
                )

        def out_tile(p, bb, xt):
            return out_pool.tile([128, O, SP], BF16, name="sb_out")

        def ship(p, bb, xt, sb_out):
            nc.sync.dma_start(
                d["out"][2 * p + bb, xt * 128:(xt + 1) * 128], sb_out[:]
            )

        # ---- software pipeline ----
        t1T_0 = proj(0)
        for o in range(O):
            tut_step(0, t1T_0, o)
        t1T_1 = proj(1)
        # B(0) finals interleaved with pair 1's tUT build: the PE never
        # waits on pair 0's evacuations, and B(0)'s DVE adds overlap
        # pair 1's PE time.
        tiles0 = [(bb, xt) for bb in range(2) for xt in range(2)]
        outs0 = {}
        seq = []
        for i, bx in enumerate(tiles0):
            seq.extend(("f", bx, c) for c in range(5))
        tut_slots = list(range(O))
        merged = []
        for i, s in enumerate(seq):
            merged.append(s)
            if i % 2 == 1 and tut_slots:
                merged.append(("t", tut_slots.pop(0)))
        for s in merged:
            if s[0] == "f":
                _, (bb, xt), c = s
                if c == 0:
                    outs0[(bb, xt)] = out_tile(0, bb, xt)
                final_chunk(0, bb, xt, c, outs0[(bb, xt)])
                if c == 4:
                    ship(0, bb, xt, outs0[(bb, xt)])
            else:
                tut_step(1, t1T_1, s[1])
        # B(1)
        for bb in range(2):
            for xt in range(2):
                is_last = bb == 1 and xt == 1
                sb_out = out_tile(1, bb, xt)
                for c in range(5):
                    final_chunk(1, bb, xt, c, sb_out)
                    if is_last and c == 2:
                        # tail: ship the finished 3/5 early
                        nc.sync.dma_start(
                            d["out"][2 + bb, xt * 128:(xt + 1) * 128, 0:6],
                            sb_out[:, 0:6, :],
                        )
                if is_last:
                    nc.sync.dma_start(
                        d["out"][2 + bb, xt * 128:(xt + 1) * 128, 6:],
                        sb_out[:, 6:, :],
                    )
                else:
                    ship(1, bb, xt, sb_out)


def build_nc():
    if "nc" in _CACHE:
        return _CACHE["nc"]
    nc = bacc.Bacc(
        "TRN2", target_bir_lowering=False, debug=False, num_devices=NCORES
    )
    d = {}
    d["stateT"] = nc.dram_tensor(
        "stateT", [NP, KT, 128, SP2], BF16, kind="ExternalInput"
    ).ap()
    d["hw"] = nc.dram_tensor("hw", [128, KT * BSE], BF16, kind="ExternalInput").ap()
    d["tw"] = nc.dram_tensor("tw", [128, KT * BSE], BF16, kind="ExternalInput").ap()
    d["ut"] = nc.dram_tensor("ut", [BSE, O * BSE], BF16, kind="ExternalInput").ap()
    d["bias"] = nc.dram_tensor("bias", [BSE, 2], F32, kind="ExternalInput").ap()
    d["cmat0"] = nc.dram_tensor(
        "cmat0", [128, O, SP], BF16, kind="ExternalInput"
    ).ap()
    d["cmat1"] = nc.dram_tensor(
        "cmat1", [128, O, SP], BF16, kind="ExternalInput"
    ).ap()
    # output layout [b][x][o][y]; host transposes (o,y)->(y,o)
    d["out"] = nc.dram_tensor(
        "out", [BPC, SP, O, SP], BF16, kind="ExternalOutput"
    ).ap()

    with tile.TileContext(nc) as tc:
        _emit(tc, d)
    nc.compile()
    _CACHE["nc"] = nc
    return nc


def prep_inputs(inputs):
    """Host-side constant packing + state transpose. Returns dict of np arrays
    shared across cores (stateT is full-batch; shard before dispatch)."""
    bf16 = ml_dtypes.bfloat16
    state = np.asarray(inputs["state"], np.float32)
    head_w = np.asarray(inputs["head_w"], np.float32)
    head_b = np.asarray(inputs["head_b"], np.float32)
    tail_w = np.asarray(inputs["tail_w"], np.float32)
    tail_b = np.asarray(inputs["tail_b"], np.float32)
    U = np.asarray(inputs["U"], np.float32)
    width_table = np.asarray(inputs["width_table"], np.float32)
    cls_w = np.asarray(inputs["cls_w"], np.float32)
    cls_b = np.asarray(inputs["cls_b"], np.float32)

    # stateT pack: [B/2, KT, 128, (b01, y)], y zero-padded to 256
    stateT = np.zeros((B, H, SP), np.float32)
    stateT[:, :, :S] = state.transpose(0, 2, 1)
    # [B/2, 2, KT, 128, SP] -> [B/2, KT, 128, 2, SP]
    stateT = stateT.reshape(B // 2, 2, KT, 128, SP).transpose(0, 2, 3, 1, 4)
    stateT = np.ascontiguousarray(
        stateT.reshape(B // 2, KT, 128, SP2)
    ).astype(bf16)

    hw_sb = np.zeros((128, KT, BSE), np.float32)
    hw_sb[:, :, :BS] = head_w.reshape(KT, 128, BS).transpose(1, 0, 2)
    hw_sb = hw_sb.reshape(128, KT * BSE).astype(bf16)
    tw_sb = np.zeros((128, KT, BSE), np.float32)
    tw_sb[:, :, :BS] = tail_w.reshape(KT, 128, BS).transpose(1, 0, 2)
    tw_sb = tw_sb.reshape(128, KT * BSE).astype(bf16)

    # ut blocks: [j, o, i] = U[o, i, j]; Wt in col i=BS; Wh folded into the
    # ones-row j=BS; cls_b folded into the (j=BS, i=BS) corner.
    blocks = np.zeros((BSE, O, BSE), np.float32)
    blocks[:BS, :, :BS] = U.transpose(2, 0, 1)
    blocks[:, :, BS] = cls_w[:, BS + 1:2 * (BS + 1)].T
    blocks[BS, :, :] += cls_w[:, :BSE]
    blocks[BS, :, BS] += cls_b
    ut = blocks.reshape(BSE, O * BSE).astype(bf16)

    bias = np.zeros((BSE, 2), np.float32)
    bias[:BS, 0] = head_b
    bias[:BS, 1] = tail_b
    bias[BS, :] = 1.0

    # cmat in [x, o, y] layout (cls_b excluded -> zero on y <= x-2):
    # cmat[x, o, y] = (width_table @ Ww.T)[pos(x,y), o]
    pos = np.arange(S)[None, :] - np.arange(S)[:, None] + 1
    pos = pos * (pos > 0)
    cvals = width_table @ cls_w[:, 2 * (BS + 1):].T        # [256, 10], row0=0
    cfull = cvals[pos].transpose(0, 2, 1).astype(bf16)      # [255, 10, 255]
    cmat0 = np.zeros((128, O, SP), bf16)
    cmat0[:, :, :S] = cfull[:128]
    cmat1 = np.zeros((128, O, SP), bf16)
    cmat1[:S - 128, :, :S] = cfull[128:]

    return {
        "stateT": stateT,
        "hw": hw_sb,
        "tw": tw_sb,
        "ut": ut,
        "bias": bias,
        "cmat0": cmat0,
        "cmat1": cmat1,
    }


def run(inputs, trace=False, trace_kwargs=None):
    nc = build_nc()
    full = prep_inputs(inputs)
    shared = {k: v for k, v in full.items() if k != "stateT"}
    in_maps = []
    for c in range(NCORES):
        m = dict(shared)
        m["stateT"] = np.ascontiguousarray(full["stateT"][c * NP:(c + 1) * NP])
        in_maps.append(m)
    res = run_bass_kernel_spmd(
        nc,
        in_maps,
        core_ids=list(range(NCORES)),
        trace=trace,
        **(trace_kwargs or {}),
    )
    # [B, x 256, o 10, y 256] bf16 -> [B, S, S, O] f32
    out = np.concatenate([np.asarray(r["out"]) for r in res.results], axis=0)
    out = out[:, :S, :, :S].transpose(0, 1, 3, 2).astype(np.float32)
    return out, res


def kernel(**inputs):
    out, _ = run(inputs, trace=False)
    return out


if __name__ == "__main__":
    build_nc()
    print("build ok")
